# revision 4
# baseline (speedup 1.0000x reference)
"""Bass/Tile kernel for bidirectional multi-head self-attention on 8 trn2 cores.

Problem: x[4, 2048, 1024], W_qkv[3072, 1024], W_proj[1024, 1024], H=16 heads,
Dh=64.  out = proj(softmax(q k^T / sqrt(Dh)) v).

Sharding: core c = (batch b = c//2, head-group g = c%2).  Each core computes
attention for 8 heads of one batch and a full-T partial output projection
(contraction over its 512 C_in columns); host sums the pair partials
(tensor-parallel unshard) and stacks batches.

v2 design (vs baseline):
  - x is pre-transposed on the host (xT [D, T]) so phase 1 needs no PE
    transposes and no DVE staging copies.
  - scores matmuls for the two heads of a pair are issued adjacently with
    lhsT/rhs on partition ranges 0:64 / 64:128 -> auto tile_position (0,0)
    and (64,0); the PE runs them concurrently in separate row groups,
    halving score time (K=64 only half-fills the array otherwise).
  - softmax exp is split between ScalarE (exact table exp) and the DVE
    (Schraudolph bf16 exp: bits = round(128*(log2e*s/8 + 127 - c)) written
    as int16 and bit-punned to bf16), keeping the exp off the critical
    path.  DVE share is a tunable fraction.
  - normalization divides ps_y rows 0:64 by the ones-row denominator
    (row 64) via a partition-broadcast DMA + reciprocal + multiply.
  - output projection runs per q-half (overlaps the second attention half)
    and stores bf16; host accumulates the pair partials in fp32.
"""

import os
import numpy as np
import ml_dtypes

import concourse.bass as bass
import concourse.bacc as bacc
import concourse.mybir as mybir
import concourse.tile as tile
from concourse.bass_utils import run_bass_kernel_spmd

# ---- problem constants (hardcoded per harness contract) --------------------
B = 4
T = 2048
D = 1024
H = 16
DH = 64
N_CORES = 8
HPC = H // 2          # heads per core = 8
F = HPC * DH          # 512 = per-core q/k/v feature width

NT = T // 128         # 16 t-tiles
NCC = D // 128        # 8 contraction chunks over D
QH = T // 2           # 1024 q-half span in attention

F32 = mybir.dt.float32
BF16 = mybir.dt.bfloat16
I16 = mybir.dt.int16

DT = BF16
NP_DT = ml_dtypes.bfloat16

# Schraudolph bf16 exp approximation: for score s (pre-scale), weight is
# exp(s/8) ~= bitcast_bf16(int16(round(s*SCH_A + SCH_B))).
SCH_C = 0.0587
SCH_A = 128.0 * 1.4426950408889634 / 8.0
SCH_B = 128.0 * (127.0 - SCH_C)

# Fraction of exp tiles computed on the DVE (Schraudolph) instead of the
# scalar engine: tiles with (idx * DVE_NUM) % DVE_DEN < DVE_NUM go to DVE.
DVE_NUM = 2
DVE_DEN = 7

LAST_EXEC_NS = None
LAST_RESULTS = None


def _exp_on_dve(idx):
    return (idx * DVE_NUM) % DVE_DEN < DVE_NUM


def build_program():
    nc = bacc.Bacc()

    xt_d = nc.dram_tensor("x_t", [D, T], DT, kind="ExternalInput")
    wqkv_d = nc.dram_tensor("w_qkv_t", [D, 3 * F], DT, kind="ExternalInput")
    wproj_d = nc.dram_tensor("w_proj_t", [F, D], DT, kind="ExternalInput")
    out_d = nc.dram_tensor("out_p", [T, D], DT, kind="ExternalOutput")

    with tile.TileContext(nc) as tc:
        with (
            tc.tile_pool(name="xw_pool", bufs=1) as xw_pool,
            tc.tile_pool(name="qk_pool", bufs=1) as qk_pool,
            tc.tile_pool(name="v_pool", bufs=1) as v_pool,
            tc.tile_pool(name="y_pool", bufs=1) as y_pool,
            tc.tile_pool(name="wp_pool", bufs=1) as wp_pool,
            tc.tile_pool(name="sc_pool", bufs=1) as sc_pool,
            tc.tile_pool(name="dram_pool", bufs=2, space="DRAM") as dram_pool,
            tc.tile_pool(name="psum", bufs=1, space="PSUM") as psum,
        ):
            # persistent tensors
            xt_sb = [xw_pool.tile([128, T], DT, name=f"xt{cc}")
                     for cc in range(NCC)]
            w_sb = [xw_pool.tile([128, 3 * F], DT, name=f"wqkv{cc}")
                    for cc in range(NCC)]
            # qkT[f]: f 0..3 -> qT for head pair f, f 4..7 -> kT head pair f-4
            qkT = [qk_pool.tile([128, T], DT, name=f"qkT{f}") for f in range(8)]
            # v_aug[tt]: [128 t, 8 heads * 65]; col 64 of each head block = 1.0
            v_aug = [v_pool.tile([128, HPC * 65], DT, name=f"vaug{t}")
                     for t in range(NT)]
            # yT[hp]: [128 dh (2 heads), T] -- normalized attention output
            yT = [y_pool.tile([128, T], DT, name=f"yT{hp}") for hp in range(4)]
            # W_proj^T slice tiles [128 dh, D]
            wp_sb = [wp_pool.tile([128, D], DT, name=f"wp{i}") for i in range(4)]

            # interleave DMAs so early c-chunks complete first
            for cc in range(NCC):
                nc.sync.dma_start(out=xt_sb[cc],
                                  in_=xt_d[cc * 128:(cc + 1) * 128, :])
                nc.sync.dma_start(out=w_sb[cc],
                                  in_=wqkv_d[cc * 128:(cc + 1) * 128, :])
            for i in range(4):
                nc.sync.dma_start(out=wp_sb[i],
                                  in_=wproj_d[i * 128:(i + 1) * 128, :])

            # ---------------- phase 1: qkv projection ----------------------
            for tci in range(4):
                t0 = tci * 512
                for f in range(8):
                    ps_qk = psum.tile([128, 512], F32, name="ps_s",
                                      tag="ps_s", bufs=4)
                    for cc in range(NCC):
                        nc.tensor.matmul(
                            ps_qk,
                            lhsT=w_sb[cc][:, f * 128:(f + 1) * 128],
                            rhs=xt_sb[cc][:, t0:t0 + 512],
                            start=(cc == 0), stop=(cc == NCC - 1))
                    nc.scalar.activation(
                        qkT[f][:, t0:t0 + 512], ps_qk,
                        mybir.ActivationFunctionType.Copy)
                for st in range(4):
                    ps_v = psum.tile([128, 512], F32, name="ps_s",
                                     tag="ps_s", bufs=4)
                    for cc in range(NCC):
                        nc.tensor.matmul(
                            ps_v,
                            lhsT=xt_sb[cc][:, t0 + st * 128:
                                           t0 + (st + 1) * 128],
                            rhs=w_sb[cc][:, 2 * F:3 * F],
                            start=(cc == 0), stop=(cc == NCC - 1))
                    va = v_aug[tci * 4 + st]
                    va_v = va.rearrange("p (h d) -> p h d", h=HPC)
                    nc.vector.tensor_copy(
                        va_v[:, :, 0:64],
                        ps_v.rearrange("p (h d) -> p h d", h=HPC))
                    nc.vector.memset(va_v[:, :, 64:65], 1.0)

            # ---------------- phase 2 + 3 interleaved ----------------------
            exp_idx = 0
            for qh in range(2):
                q0 = qh * QH
                for hp in range(4):
                    qT = qkT[hp]
                    kT = qkT[4 + hp]
                    ps_y = [psum.tile([65, QH], F32, name=f"ps_y{hh}",
                                      tag=f"ps_y{hh}", bufs=1)
                            for hh in range(2)]
                    h0 = 2 * hp
                    for kt in range(NT):
                        for qc in range(2):
                            ps_sc = []
                            att = []
                            for hh in range(2):
                                ps_sc.append(
                                    psum.tile([128, 512], F32, name="ps_s",
                                              tag="ps_s", bufs=4))
                                att.append(
                                    sc_pool.tile([128, 512], DT, name="att",
                                                 tag=f"att{hh}", bufs=2))
                            # scores: adjacent row-group matmuls overlap
                            for hh in range(2):
                                r0 = hh * 64
                                nc.tensor.matmul(
                                    ps_sc[hh],
                                    lhsT=kT[r0:r0 + 64,
                                            kt * 128:(kt + 1) * 128],
                                    rhs=qT[r0:r0 + 64,
                                           q0 + qc * 512:q0 + (qc + 1) * 512],
                                    start=True, stop=True)
                            for hh in range(2):
                                if _exp_on_dve(exp_idx):
                                    nc.vector.tensor_scalar(
                                        out=att[hh].bitcast(I16),
                                        in0=ps_sc[hh],
                                        scalar1=SCH_A, scalar2=SCH_B,
                                        op0=mybir.AluOpType.mult,
                                        op1=mybir.AluOpType.add)
                                else:
                                    nc.scalar.activation(
                                        att[hh], ps_sc[hh],
                                        mybir.ActivationFunctionType.Exp,
                                        scale=1.0 / 8.0)
                                exp_idx += 1
                            for hh in range(2):
                                nc.tensor.matmul(
                                    ps_y[hh][:, qc * 512:(qc + 1) * 512],
                                    lhsT=v_aug[kt][:, (h0 + hh) * 65:
                                                   (h0 + hh) * 65 + 65],
                                    rhs=att[hh],
                                    start=(kt == 0), stop=(kt == NT - 1))
                    # normalization: denominator row 64 -> DRAM -> partition
                    # broadcast -> reciprocal -> multiply rows 0:64
                    for hh in range(2):
                        d_sb = sc_pool.tile([65, QH], F32, name="d_sb",
                                            tag="d_sb", bufs=2)
                        nc.scalar.copy(d_sb[64:65, :], ps_y[hh][64:65, :])
                        d_dram = dram_pool.tile([1, QH], F32, name="d_dram",
                                                tag="d_dram")
                        nc.sync.dma_start(out=d_dram, in_=d_sb[64:65, :])
                        d_bc = sc_pool.tile([64, QH], F32, name="d_bc",
                                            tag="d_bc", bufs=2)
                        nc.sync.dma_start(
                            out=d_bc,
                            in_=bass.AP(tensor=d_dram.tensor,
                                        offset=d_dram.offset,
                                        ap=[[0, 64]] + list(d_dram.ap[1:])))
                        r_bc = sc_pool.tile([64, QH], F32, name="r_bc",
                                            tag="r_bc", bufs=2)
                        nc.vector.reciprocal_approx_fast(r_bc, d_bc)
                        y_tmp = sc_pool.tile([64, QH], DT, name="y_tmp",
                                             tag="y_tmp", bufs=2)
                        nc.vector.tensor_mul(y_tmp, ps_y[hh][0:64, :], r_bc)
                        nc.sync.dma_start(
                            out=yT[hp][hh * 64:(hh + 1) * 64, q0:q0 + QH],
                            in_=y_tmp)
                # phase 3 for this q-half's t-tiles
                for tt in range(qh * 8, qh * 8 + 8):
                    o_sb = sc_pool.tile([128, D], DT, name="o_sb",
                                        tag="o_sb", bufs=3)
                    for oc in range(2):
                        ps_o = psum.tile([128, 512], F32, name="ps_s",
                                         tag="ps_s", bufs=4)
                        for hp in range(4):
                            nc.tensor.matmul(
                                ps_o,
                                lhsT=yT[hp][:, tt * 128:(tt + 1) * 128],
                                rhs=wp_sb[hp][:, oc * 512:(oc + 1) * 512],
                                start=(hp == 0), stop=(hp == 3))
                        nc.scalar.activation(
                            o_sb[:, oc * 512:(oc + 1) * 512], ps_o,
                            mybir.ActivationFunctionType.Copy)
                    nc.sync.dma_start(out=out_d[tt * 128:(tt + 1) * 128, :],
                                      in_=o_sb)
    return nc


_NC_CACHE = None


def _get_program():
    global _NC_CACHE
    if _NC_CACHE is None:
        nc = build_program()
        if not nc.is_finalized():
            nc.finalize()
        _NC_CACHE = nc
    return _NC_CACHE


def make_in_maps(x, W_qkv, W_proj):
    """Shard full inputs into per-core input maps (host-side layout prep)."""
    Wq, Wk, Wv = W_qkv[0:D], W_qkv[D:2 * D], W_qkv[2 * D:3 * D]
    maps = []
    xt_b, wq_g, wp_g = {}, {}, {}
    for b in range(B):
        xt_b[b] = np.ascontiguousarray(x[b].T).astype(NP_DT)
    for g in range(2):
        rows = slice(g * F, (g + 1) * F)
        wq_g[g] = np.ascontiguousarray(
            np.concatenate([Wq[rows].T, Wk[rows].T, Wv[rows].T], axis=1)
        ).astype(NP_DT)
        wp_g[g] = np.ascontiguousarray(W_proj[:, rows].T).astype(NP_DT)
    for core in range(N_CORES):
        b, g = core // 2, core % 2
        maps.append({
            "x_t": xt_b[b],
            "w_qkv_t": wq_g[g],
            "w_proj_t": wp_g[g],
        })
    return maps


def kernel(x, W_qkv, W_proj):
    global LAST_EXEC_NS, LAST_RESULTS
    x = np.asarray(x, dtype=np.float32)
    W_qkv = np.asarray(W_qkv, dtype=np.float32)
    W_proj = np.asarray(W_proj, dtype=np.float32)

    nc = _get_program()
    in_maps = make_in_maps(x, W_qkv, W_proj)
    trace = bool(int(os.environ.get("BASS_KERNEL_TRACE", "0")))
    res = run_bass_kernel_spmd(nc, in_maps, list(range(N_CORES)), trace=trace)
    LAST_EXEC_NS = res.exec_time_ns
    LAST_RESULTS = res
    out = np.stack([
        np.asarray(res.results[2 * b]["out_p"], dtype=np.float32)
        + np.asarray(res.results[2 * b + 1]["out_p"], dtype=np.float32)
        for b in range(B)
    ])
    return out


# revision 5
# speedup vs baseline: 1.1020x; 1.1020x over previous
"""Bass/Tile kernel for bidirectional multi-head self-attention on 8 trn2 cores.

Problem: x[4, 2048, 1024], W_qkv[3072, 1024], W_proj[1024, 1024], H=16 heads,
Dh=64.  out = proj(softmax(q k^T / sqrt(Dh)) v).

Sharding: core c = (batch b = c//2, head-group g = c%2).  Each core computes
attention for 8 heads of one batch and a full-T partial output projection
(contraction over its 512 C_in columns); host sums the pair partials
(tensor-parallel unshard) and stacks batches.

v3 design:
  - x is pre-transposed on the host (xT [D, T]): phase 1 needs no PE
    transposes or staging copies.
  - scores matmuls for the two heads of a pair sit on partition ranges
    0:64 / 64:128 -> auto tile_position (0,0)/(64,0); issued adjacently
    they run concurrently in separate PE row groups (K=64 would otherwise
    half-fill the array).
  - all elementwise work runs at [128, 1024] grain (the ~300ns fixed
    per-op engine overhead is 30%+ at FD=512): scores PSUM tiles span a
    kt-pair, phase 1/3 PSUM tiles span two 512 outputs.
  - softmax exp is split between ScalarE (exact) and DVE (Schraudolph
    bf16 exp: int16(round(s*A + B)) bit-punned to bf16).
  - ps_y is effectively double buffered (one tag, 4 bufs) so the
    normalization chain (denominator row -> DRAM -> partition-broadcast
    -> reciprocal -> multiply) never stalls the PE: stalls > 3.4us would
    re-throttle the PE clock to 1.2GHz (HAM), which is what sank v2.
  - the output projection runs per q-chunk, overlapping attention.
"""

import os
import numpy as np
import ml_dtypes

import concourse.bass as bass
import concourse.bacc as bacc
import concourse.mybir as mybir
import concourse.tile as tile
from concourse.bass_utils import run_bass_kernel_spmd

# ---- problem constants (hardcoded per harness contract) --------------------
B = 4
T = 2048
D = 1024
H = 16
DH = 64
N_CORES = 8
HPC = H // 2          # heads per core = 8
F = HPC * DH          # 512 = per-core q/k/v feature width

NT = T // 128         # 16 k-tiles
NCC = D // 128        # 8 contraction chunks over D
QC = 512              # q-chunk
NQC = T // QC         # 4 q-chunks

F32 = mybir.dt.float32
BF16 = mybir.dt.bfloat16
I16 = mybir.dt.int16

DT = BF16
NP_DT = ml_dtypes.bfloat16

# Schraudolph bf16 exp approximation: for raw score s, weight is
# exp(s/8) ~= bitcast_bf16(int16(round(s*SCH_A + SCH_B))).
SCH_C = 0.0587
SCH_A = 128.0 * 1.4426950408889634 / 8.0
SCH_B = 128.0 * (127.0 - SCH_C)

# Fraction of exp tiles computed on the DVE (Schraudolph) instead of
# ScalarE: tile idx goes to DVE when (idx * DVE_NUM) % DVE_DEN < DVE_NUM.
DVE_NUM = 3
DVE_DEN = 8

LAST_EXEC_NS = None
LAST_RESULTS = None


def _exp_on_dve(idx):
    return (idx * DVE_NUM) % DVE_DEN < DVE_NUM


def build_program():
    nc = bacc.Bacc()

    xt_d = nc.dram_tensor("x_t", [D, T], DT, kind="ExternalInput")
    wqkv_d = nc.dram_tensor("w_qkv_t", [D, 3 * F], DT, kind="ExternalInput")
    wproj_d = nc.dram_tensor("w_proj_t", [F, D], DT, kind="ExternalInput")
    out_d = nc.dram_tensor("out_p", [T, D], DT, kind="ExternalOutput")

    with tile.TileContext(nc) as tc:
        with (
            tc.tile_pool(name="xw_pool", bufs=1) as xw_pool,
            tc.tile_pool(name="qk_pool", bufs=1) as qk_pool,
            tc.tile_pool(name="v_pool", bufs=1) as v_pool,
            tc.tile_pool(name="y_pool", bufs=1) as y_pool,
            tc.tile_pool(name="wp_pool", bufs=1) as wp_pool,
            tc.tile_pool(name="sc_pool", bufs=1) as sc_pool,
            tc.tile_pool(name="dram_pool", bufs=2, space="DRAM") as dram_pool,
            tc.tile_pool(name="psum", bufs=1, space="PSUM") as psum,
        ):
            # persistent tensors
            xt_sb = [xw_pool.tile([128, T], DT, name=f"xt{cc}")
                     for cc in range(NCC)]
            w_sb = [xw_pool.tile([128, 3 * F], DT, name=f"wqkv{cc}")
                    for cc in range(NCC)]
            # qkT[f]: f 0..3 -> qT for head pair f, f 4..7 -> kT head pair f-4
            qkT = [qk_pool.tile([128, T], DT, name=f"qkT{f}") for f in range(8)]
            # v_aug[i]: two k-tiles [128 t, 2*(8 heads*65)]; col 64 of each
            # head block is 1.0 (softmax denominator via the AV matmul)
            v_aug = [v_pool.tile([128, 2 * HPC * 65], DT, name=f"vaug{i}")
                     for i in range(NT // 2)]
            # yT[hp]: [128 dh (2 heads), T] -- normalized attention output
            yT = [y_pool.tile([128, T], DT, name=f"yT{hp}") for hp in range(4)]
            # W_proj^T slice tiles [128 dh, D]
            wp_sb = [wp_pool.tile([128, D], DT, name=f"wp{i}") for i in range(4)]

            for cc in range(NCC):
                nc.sync.dma_start(out=xt_sb[cc],
                                  in_=xt_d[cc * 128:(cc + 1) * 128, :])
                nc.sync.dma_start(out=w_sb[cc],
                                  in_=wqkv_d[cc * 128:(cc + 1) * 128, :])
            for i in range(4):
                nc.sync.dma_start(out=wp_sb[i],
                                  in_=wproj_d[i * 128:(i + 1) * 128, :])

            # ---------------- phase 1: qkv projection ----------------------
            for tcp in range(2):   # t-chunk pairs (1024 t positions)
                t0 = tcp * 1024
                for f in range(8):
                    ps = psum.tile([128, 1024], F32, name="ps_big",
                                   tag="ps_big", bufs=2)
                    for half in range(2):
                        for cc in range(NCC):
                            nc.tensor.matmul(
                                ps[:, half * 512:(half + 1) * 512],
                                lhsT=w_sb[cc][:, f * 128:(f + 1) * 128],
                                rhs=xt_sb[cc][:, t0 + half * 512:
                                              t0 + (half + 1) * 512],
                                start=(cc == 0), stop=(cc == NCC - 1))
                    nc.scalar.activation(
                        qkT[f][:, t0:t0 + 1024], ps,
                        mybir.ActivationFunctionType.Copy)
                for sv in range(4):  # pairs of t-tiles -> one v_aug tile
                    ps = psum.tile([128, 1024], F32, name="ps_big",
                                   tag="ps_big", bufs=2)
                    for half in range(2):
                        tt0 = t0 + sv * 256 + half * 128
                        for cc in range(NCC):
                            nc.tensor.matmul(
                                ps[:, half * 512:(half + 1) * 512],
                                lhsT=xt_sb[cc][:, tt0:tt0 + 128],
                                rhs=w_sb[cc][:, 2 * F:3 * F],
                                start=(cc == 0), stop=(cc == NCC - 1))
                    va = v_aug[tcp * 4 + sv]
                    va_v = va.rearrange("p (k h d) -> p k h d", k=2, h=HPC)
                    nc.vector.tensor_copy(
                        va_v[:, :, :, 0:64],
                        ps.rearrange("p (k h d) -> p k h d", k=2, h=HPC))
                    nc.vector.memset(va_v[:, :, :, 64:65], 1.0)

            # ---------------- phase 2 + 3 interleaved ----------------------
            exp_idx = 0
            d_idx = 0
            for qc in range(NQC):
                q0 = qc * QC
                for hp in range(4):
                    qT = qkT[hp]
                    kT = qkT[4 + hp]
                    h0 = 2 * hp
                    ps_y = [psum.tile([65, QC], F32, name="ps_y",
                                      tag="ps_y", bufs=4) for _ in range(2)]
                    for ktp in range(8):
                        ps2 = []
                        att2 = []
                        for hh in range(2):
                            ps2.append(psum.tile([128, 1024], F32,
                                                 name="ps_big", tag="ps_big",
                                                 bufs=2))
                            att2.append(sc_pool.tile([128, 1024], DT,
                                                     name="att",
                                                     tag=f"att{hh}", bufs=2))
                        # scores for kt pair; adjacent row-group matmuls
                        # (partitions 0:64 vs 64:128) overlap on the PE
                        for k2 in range(2):
                            kt = 2 * ktp + k2
                            for hh in range(2):
                                r0 = hh * 64
                                nc.tensor.matmul(
                                    ps2[hh][:, k2 * 512:(k2 + 1) * 512],
                                    lhsT=kT[r0:r0 + 64,
                                            kt * 128:(kt + 1) * 128],
                                    rhs=qT[r0:r0 + 64, q0:q0 + QC],
                                    start=True, stop=True)
                        for hh in range(2):
                            if _exp_on_dve(exp_idx):
                                nc.vector.tensor_scalar(
                                    out=att2[hh].bitcast(I16),
                                    in0=ps2[hh],
                                    scalar1=SCH_A, scalar2=SCH_B,
                                    op0=mybir.AluOpType.mult,
                                    op1=mybir.AluOpType.add)
                            else:
                                nc.scalar.activation(
                                    att2[hh], ps2[hh],
                                    mybir.ActivationFunctionType.Exp,
                                    scale=1.0 / 8.0)
                            exp_idx += 1
                        for k2 in range(2):
                            kt = 2 * ktp + k2
                            for hh in range(2):
                                nc.tensor.matmul(
                                    ps_y[hh],
                                    lhsT=v_aug[ktp][:,
                                                    (k2 * HPC + h0 + hh) * 65:
                                                    (k2 * HPC + h0 + hh) * 65
                                                    + 65],
                                    rhs=att2[hh][:, k2 * 512:(k2 + 1) * 512],
                                    start=(kt == 0), stop=(kt == NT - 1))
                    # normalization: denominator row 64 -> DRAM ->
                    # partition-broadcast -> reciprocal -> multiply
                    for hh in range(2):
                        d_sb = sc_pool.tile([65, QC], F32, name="d_sb",
                                            tag="d_sb", bufs=2)
                        if d_idx % 2 == 0:
                            nc.scalar.copy(d_sb[64:65, :], ps_y[hh][64:65, :])
                        else:
                            nc.vector.tensor_copy(d_sb[64:65, :],
                                                  ps_y[hh][64:65, :])
                        d_idx += 1
                        d_dram = dram_pool.tile([1, QC], F32, name="d_dram",
                                                tag="d_dram")
                        nc.sync.dma_start(out=d_dram, in_=d_sb[64:65, :])
                        d_bc = sc_pool.tile([64, QC], F32, name="d_bc",
                                            tag="d_bc", bufs=2)
                        nc.sync.dma_start(
                            out=d_bc,
                            in_=bass.AP(tensor=d_dram.tensor,
                                        offset=d_dram.offset,
                                        ap=[[0, 64]] + list(d_dram.ap[1:])))
                        r_bc = sc_pool.tile([64, QC], F32, name="r_bc",
                                            tag="r_bc", bufs=2)
                        nc.vector.reciprocal_approx_fast(r_bc, d_bc)
                        y_tmp = sc_pool.tile([64, QC], DT, name="y_tmp",
                                             tag="y_tmp", bufs=2)
                        nc.vector.tensor_mul(y_tmp, ps_y[hh][0:64, :], r_bc)
                        nc.sync.dma_start(
                            out=yT[hp][hh * 64:(hh + 1) * 64, q0:q0 + QC],
                            in_=y_tmp)
                # phase 3 for this q-chunk's t-tiles
                for tt in range(qc * 4, qc * 4 + 4):
                    ps_o = psum.tile([128, 1024], F32, name="ps_big",
                                     tag="ps_big", bufs=2)
                    for oc in range(2):
                        for hp in range(4):
                            nc.tensor.matmul(
                                ps_o[:, oc * 512:(oc + 1) * 512],
                                lhsT=yT[hp][:, tt * 128:(tt + 1) * 128],
                                rhs=wp_sb[hp][:, oc * 512:(oc + 1) * 512],
                                start=(hp == 0), stop=(hp == 3))
                    o_sb = sc_pool.tile([128, D], DT, name="o_sb",
                                        tag="o_sb", bufs=3)
                    nc.scalar.activation(o_sb, ps_o,
                                         mybir.ActivationFunctionType.Copy)
                    nc.sync.dma_start(out=out_d[tt * 128:(tt + 1) * 128, :],
                                      in_=o_sb)
    return nc


_NC_CACHE = None


def _get_program():
    global _NC_CACHE
    if _NC_CACHE is None:
        nc = build_program()
        if not nc.is_finalized():
            nc.finalize()
        _NC_CACHE = nc
    return _NC_CACHE


def make_in_maps(x, W_qkv, W_proj):
    """Shard full inputs into per-core input maps (host-side layout prep)."""
    Wq, Wk, Wv = W_qkv[0:D], W_qkv[D:2 * D], W_qkv[2 * D:3 * D]
    maps = []
    xt_b, wq_g, wp_g = {}, {}, {}
    for b in range(B):
        xt_b[b] = np.ascontiguousarray(x[b].T).astype(NP_DT)
    for g in range(2):
        rows = slice(g * F, (g + 1) * F)
        wq_g[g] = np.ascontiguousarray(
            np.concatenate([Wq[rows].T, Wk[rows].T, Wv[rows].T], axis=1)
        ).astype(NP_DT)
        wp_g[g] = np.ascontiguousarray(W_proj[:, rows].T).astype(NP_DT)
    for core in range(N_CORES):
        b, g = core // 2, core % 2
        maps.append({
            "x_t": xt_b[b],
            "w_qkv_t": wq_g[g],
            "w_proj_t": wp_g[g],
        })
    return maps


def kernel(x, W_qkv, W_proj):
    global LAST_EXEC_NS, LAST_RESULTS
    x = np.asarray(x, dtype=np.float32)
    W_qkv = np.asarray(W_qkv, dtype=np.float32)
    W_proj = np.asarray(W_proj, dtype=np.float32)

    nc = _get_program()
    in_maps = make_in_maps(x, W_qkv, W_proj)
    trace = bool(int(os.environ.get("BASS_KERNEL_TRACE", "0")))
    res = run_bass_kernel_spmd(nc, in_maps, list(range(N_CORES)), trace=trace)
    LAST_EXEC_NS = res.exec_time_ns
    LAST_RESULTS = res
    out = np.stack([
        np.asarray(res.results[2 * b]["out_p"], dtype=np.float32)
        + np.asarray(res.results[2 * b + 1]["out_p"], dtype=np.float32)
        for b in range(B)
    ])
    return out


# revision 8
# speedup vs baseline: 1.2050x; 1.0935x over previous
"""Bass/Tile kernel for bidirectional multi-head self-attention on 8 trn2 cores.

Problem: x[4, 2048, 1024], W_qkv[3072, 1024], W_proj[1024, 1024], H=16 heads,
Dh=64.  out = proj(softmax(q k^T / sqrt(Dh)) v).

Sharding: core c = (batch b = c//2, head-group g = c%2).  Each core computes
attention for 8 heads of one batch and a full-T partial output projection
(contraction over its 512 C_in columns); host sums the pair partials
(tensor-parallel unshard) and stacks batches.

v3 design:
  - x is pre-transposed on the host (xT [D, T]): phase 1 needs no PE
    transposes or staging copies.
  - scores matmuls for the two heads of a pair sit on partition ranges
    0:64 / 64:128 -> auto tile_position (0,0)/(64,0); issued adjacently
    they run concurrently in separate PE row groups (K=64 would otherwise
    half-fill the array).
  - all elementwise work runs at [128, 1024] grain (the ~300ns fixed
    per-op engine overhead is 30%+ at FD=512): scores PSUM tiles span a
    kt-pair, phase 1/3 PSUM tiles span two 512 outputs.
  - softmax exp is split between ScalarE (exact) and DVE (Schraudolph
    bf16 exp: int16(round(s*A + B)) bit-punned to bf16).
  - ps_y is effectively double buffered (one tag, 4 bufs) so the
    normalization chain (denominator row -> DRAM -> partition-broadcast
    -> reciprocal -> multiply) never stalls the PE: stalls > 3.4us would
    re-throttle the PE clock to 1.2GHz (HAM), which is what sank v2.
  - the output projection runs per q-chunk, overlapping attention.
"""

import os
import numpy as np
import ml_dtypes

import concourse.bass as bass
import concourse.bacc as bacc
import concourse.mybir as mybir
import concourse.tile as tile
from concourse.bass_utils import run_bass_kernel_spmd

# ---- problem constants (hardcoded per harness contract) --------------------
B = 4
T = 2048
D = 1024
H = 16
DH = 64
N_CORES = 8
HPC = H // 2          # heads per core = 8
F = HPC * DH          # 512 = per-core q/k/v feature width

NT = T // 128         # 16 k-tiles
NCC = D // 128        # 8 contraction chunks over D
QC = 512              # q-chunk
NQC = T // QC         # 4 q-chunks

F32 = mybir.dt.float32
BF16 = mybir.dt.bfloat16
I16 = mybir.dt.int16

DT = BF16
NP_DT = ml_dtypes.bfloat16

# Schraudolph bf16 exp approximation: for raw score s, weight is
# exp(s/8) ~= bitcast_bf16(int16(round(s*SCH_A + SCH_B))).
SCH_C = 0.0587
SCH_A = 128.0 * 1.4426950408889634 / 8.0
SCH_B = 128.0 * (127.0 - SCH_C)

# Fraction of exp tiles computed on the DVE (Schraudolph) instead of
# ScalarE: tile idx goes to DVE when (idx * DVE_NUM) % DVE_DEN < DVE_NUM.
DVE_NUM = 3
DVE_DEN = 8

LAST_EXEC_NS = None
LAST_RESULTS = None


def _exp_on_dve(idx):
    return (idx * DVE_NUM) % DVE_DEN < DVE_NUM


def build_program():
    nc = bacc.Bacc()

    xt_d = nc.dram_tensor("x_t", [D, T], DT, kind="ExternalInput")
    wqkv_d = nc.dram_tensor("w_qkv_t", [D, 3 * F], DT, kind="ExternalInput")
    wproj_d = nc.dram_tensor("w_proj_t", [F, D], DT, kind="ExternalInput")
    out_d = nc.dram_tensor("out_p", [T, D], DT, kind="ExternalOutput")

    with tile.TileContext(nc) as tc:
        with (
            tc.tile_pool(name="xw_pool", bufs=1) as xw_pool,
            tc.tile_pool(name="qk_pool", bufs=1) as qk_pool,
            tc.tile_pool(name="v_pool", bufs=1) as v_pool,
            tc.tile_pool(name="y_pool", bufs=1) as y_pool,
            tc.tile_pool(name="wp_pool", bufs=1) as wp_pool,
            tc.tile_pool(name="sc_pool", bufs=1) as sc_pool,
            tc.tile_pool(name="dram_pool", bufs=2, space="DRAM") as dram_pool,
            tc.tile_pool(name="psum", bufs=1, space="PSUM") as psum,
        ):
            # persistent tensors
            xt_sb = [xw_pool.tile([128, T], DT, name=f"xt{cc}")
                     for cc in range(NCC)]
            w_sb = [xw_pool.tile([128, 3 * F], DT, name=f"wqkv{cc}")
                    for cc in range(NCC)]
            # qkT[f]: f 0..3 -> qT for head pair f, f 4..7 -> kT head pair f-4
            qkT = [qk_pool.tile([128, T], DT, name=f"qkT{f}") for f in range(8)]
            # v_aug[i]: two k-tiles [128 t, 2*(8 heads*65)]; col 64 of each
            # head block is 1.0 (softmax denominator via the AV matmul)
            v_aug = [v_pool.tile([128, 2 * HPC * 65], DT, name=f"vaug{i}")
                     for i in range(NT // 2)]
            # yT[hp]: [128 dh (2 heads), T] -- normalized attention output
            yT = [y_pool.tile([128, T], DT, name=f"yT{hp}") for hp in range(4)]
            # W_proj^T slice tiles [128 dh, D]
            wp_sb = [wp_pool.tile([128, D], DT, name=f"wp{i}") for i in range(4)]

            # chunked in consumption order so phase 1 starts ~4us in
            for ck in range(4):
                for cc in range(NCC):
                    nc.sync.dma_start(
                        out=xt_sb[cc][:, ck * 512:(ck + 1) * 512],
                        in_=xt_d[cc * 128:(cc + 1) * 128,
                                 ck * 512:(ck + 1) * 512])
                    nc.sync.dma_start(
                        out=w_sb[cc][:, ck * 384:(ck + 1) * 384],
                        in_=wqkv_d[cc * 128:(cc + 1) * 128,
                                   ck * 384:(ck + 1) * 384])
            for i in range(4):
                nc.sync.dma_start(out=wp_sb[i],
                                  in_=wproj_d[i * 128:(i + 1) * 128, :])

            # ---------------- phase 1: qkv projection ----------------------
            for tcp in range(2):   # t-chunk pairs (1024 t positions)
                t0 = tcp * 1024
                for f in range(8):
                    ps = psum.tile([128, 1024], F32, name="ps_big",
                                   tag="ps_big", bufs=2)
                    for half in range(2):
                        for cc in range(NCC):
                            nc.tensor.matmul(
                                ps[:, half * 512:(half + 1) * 512],
                                lhsT=w_sb[cc][:, f * 128:(f + 1) * 128],
                                rhs=xt_sb[cc][:, t0 + half * 512:
                                              t0 + (half + 1) * 512],
                                start=(cc == 0), stop=(cc == NCC - 1))
                    nc.scalar.activation(
                        qkT[f][:, t0:t0 + 1024], ps,
                        mybir.ActivationFunctionType.Copy)
                for sv in range(4):  # pairs of t-tiles -> one v_aug tile
                    ps = psum.tile([128, 1024], F32, name="ps_big",
                                   tag="ps_big", bufs=2)
                    for half in range(2):
                        tt0 = t0 + sv * 256 + half * 128
                        for cc in range(NCC):
                            nc.tensor.matmul(
                                ps[:, half * 512:(half + 1) * 512],
                                lhsT=xt_sb[cc][:, tt0:tt0 + 128],
                                rhs=w_sb[cc][:, 2 * F:3 * F],
                                start=(cc == 0), stop=(cc == NCC - 1))
                    va = v_aug[tcp * 4 + sv]
                    va_v = va.rearrange("p (k h d) -> p k h d", k=2, h=HPC)
                    nc.vector.tensor_copy(
                        va_v[:, :, :, 0:64],
                        ps.rearrange("p (k h d) -> p k h d", k=2, h=HPC))
                    nc.vector.memset(va_v[:, :, :, 64:65], 1.0)

            # ---------------- phase 2 + 3 interleaved ----------------------
            exp_idx = 0
            d_idx = 0
            # deferred-normalization state: emitted in two spread stages
            # inside the NEXT pair's ktp loop so the DMA round-trip latency
            # never sits at the head of an engine FIFO (head-of-line
            # blocking there stalls queued exps -> PSUM recycling -> PE,
            # and PE idle gaps > 3.4us re-throttle its clock).
            pending = None  # (qc0, hp, ps_y pair)
            staged = None   # (qc0, hp, ps_y pair, d_bc tiles)

            def norm_stage1(task):
                nonlocal d_idx
                _q0, _hp, _ps_y = task
                d_bcs = []
                for hh in range(2):
                    d_sb = sc_pool.tile([65, QC], F32, name="d_sb",
                                        tag="d_sb", bufs=2)
                    if d_idx % 2 == 0:
                        nc.scalar.copy(d_sb[64:65, :], _ps_y[hh][64:65, :])
                    else:
                        nc.vector.tensor_copy(d_sb[64:65, :],
                                              _ps_y[hh][64:65, :])
                    d_idx += 1
                    d_dram = dram_pool.tile([1, QC], F32, name="d_dram",
                                            tag="d_dram")
                    nc.sync.dma_start(out=d_dram, in_=d_sb[64:65, :])
                    d_bc = sc_pool.tile([64, QC], F32, name="d_bc",
                                        tag="d_bc", bufs=2)
                    nc.sync.dma_start(
                        out=d_bc,
                        in_=bass.AP(tensor=d_dram.tensor,
                                    offset=d_dram.offset,
                                    ap=[[0, 64]] + list(d_dram.ap[1:])))
                    d_bcs.append(d_bc)
                return (*task, d_bcs)

            def norm_stage2(task):
                _q0, _hp, _ps_y, d_bcs = task
                for hh in range(2):
                    r_bc = sc_pool.tile([64, QC], F32, name="r_bc",
                                        tag="r_bc", bufs=2)
                    nc.vector.reciprocal_approx_fast(r_bc, d_bcs[hh])
                    y_tmp = sc_pool.tile([64, QC], DT, name="y_tmp",
                                         tag="y_tmp", bufs=2)
                    nc.vector.tensor_mul(y_tmp, _ps_y[hh][0:64, :], r_bc)
                    nc.sync.dma_start(
                        out=yT[_hp][hh * 64:(hh + 1) * 64, _q0:_q0 + QC],
                        in_=y_tmp)

            def emit_proj(qcp):
                for tt in range(qcp * 4, qcp * 4 + 4):
                    ps_o = psum.tile([128, 1024], F32, name="ps_big",
                                     tag="ps_big", bufs=2)
                    for oc in range(2):
                        for hp_ in range(4):
                            nc.tensor.matmul(
                                ps_o[:, oc * 512:(oc + 1) * 512],
                                lhsT=yT[hp_][:, tt * 128:(tt + 1) * 128],
                                rhs=wp_sb[hp_][:, oc * 512:(oc + 1) * 512],
                                start=(hp_ == 0), stop=(hp_ == 3))
                    o_sb = sc_pool.tile([128, D], DT, name="o_sb",
                                        tag="o_sb", bufs=3)
                    nc.scalar.activation(o_sb, ps_o,
                                         mybir.ActivationFunctionType.Copy)
                    nc.sync.dma_start(out=out_d[tt * 128:(tt + 1) * 128, :],
                                      in_=o_sb)

            for qc in range(NQC):
                q0 = qc * QC
                for hp in range(4):
                    qT = qkT[hp]
                    kT = qkT[4 + hp]
                    h0 = 2 * hp
                    ps_y = [psum.tile([65, QC], F32, name="ps_y",
                                      tag="ps_y", bufs=4) for _ in range(2)]
                    for ktp in range(8):
                        if ktp == 1 and pending is not None:
                            staged = norm_stage1(pending)
                            pending = None
                        if ktp == 4 and staged is not None:
                            norm_stage2(staged)
                            staged = None
                        ps2 = []
                        att2 = []
                        for hh in range(2):
                            ps2.append(psum.tile([128, 1024], F32,
                                                 name="ps_big", tag="ps_big",
                                                 bufs=2))
                            att2.append(sc_pool.tile([128, 1024], DT,
                                                     name="att",
                                                     tag=f"att{hh}", bufs=2))
                        # scores for kt pair; adjacent row-group matmuls
                        # (partitions 0:64 vs 64:128) overlap on the PE
                        for k2 in range(2):
                            kt = 2 * ktp + k2
                            for hh in range(2):
                                r0 = hh * 64
                                nc.tensor.matmul(
                                    ps2[hh][:, k2 * 512:(k2 + 1) * 512],
                                    lhsT=kT[r0:r0 + 64,
                                            kt * 128:(kt + 1) * 128],
                                    rhs=qT[r0:r0 + 64, q0:q0 + QC],
                                    start=True, stop=True)
                        for hh in range(2):
                            if _exp_on_dve(exp_idx):
                                nc.vector.tensor_scalar(
                                    out=att2[hh].bitcast(I16),
                                    in0=ps2[hh],
                                    scalar1=SCH_A, scalar2=SCH_B,
                                    op0=mybir.AluOpType.mult,
                                    op1=mybir.AluOpType.add)
                            else:
                                nc.scalar.activation(
                                    att2[hh], ps2[hh],
                                    mybir.ActivationFunctionType.Exp,
                                    scale=1.0 / 8.0)
                            exp_idx += 1
                        for k2 in range(2):
                            kt = 2 * ktp + k2
                            for hh in range(2):
                                nc.tensor.matmul(
                                    ps_y[hh],
                                    lhsT=v_aug[ktp][:,
                                                    (k2 * HPC + h0 + hh) * 65:
                                                    (k2 * HPC + h0 + hh) * 65
                                                    + 65],
                                    rhs=att2[hh][:, k2 * 512:(k2 + 1) * 512],
                                    start=(kt == 0), stop=(kt == NT - 1))
                    pending = (q0, hp, ps_y)
                # output projection deferred by one q-chunk
                if qc >= 1:
                    emit_proj(qc - 1)
            # drain: last pair's normalization + last proj chunk
            staged = norm_stage1(pending)
            norm_stage2(staged)
            emit_proj(NQC - 1)
    return nc


_NC_CACHE = None


def _get_program():
    global _NC_CACHE
    if _NC_CACHE is None:
        nc = build_program()
        if not nc.is_finalized():
            nc.finalize()
        _NC_CACHE = nc
    return _NC_CACHE


def make_in_maps(x, W_qkv, W_proj):
    """Shard full inputs into per-core input maps (host-side layout prep)."""
    Wq, Wk, Wv = W_qkv[0:D], W_qkv[D:2 * D], W_qkv[2 * D:3 * D]
    maps = []
    xt_b, wq_g, wp_g = {}, {}, {}
    for b in range(B):
        xt_b[b] = np.ascontiguousarray(x[b].T).astype(NP_DT)
    for g in range(2):
        rows = slice(g * F, (g + 1) * F)
        wq_g[g] = np.ascontiguousarray(
            np.concatenate([Wq[rows].T, Wk[rows].T, Wv[rows].T], axis=1)
        ).astype(NP_DT)
        wp_g[g] = np.ascontiguousarray(W_proj[:, rows].T).astype(NP_DT)
    for core in range(N_CORES):
        b, g = core // 2, core % 2
        maps.append({
            "x_t": xt_b[b],
            "w_qkv_t": wq_g[g],
            "w_proj_t": wp_g[g],
        })
    return maps


def kernel(x, W_qkv, W_proj):
    global LAST_EXEC_NS, LAST_RESULTS
    x = np.asarray(x, dtype=np.float32)
    W_qkv = np.asarray(W_qkv, dtype=np.float32)
    W_proj = np.asarray(W_proj, dtype=np.float32)

    nc = _get_program()
    in_maps = make_in_maps(x, W_qkv, W_proj)
    trace = bool(int(os.environ.get("BASS_KERNEL_TRACE", "0")))
    res = run_bass_kernel_spmd(nc, in_maps, list(range(N_CORES)), trace=trace)
    LAST_EXEC_NS = res.exec_time_ns
    LAST_RESULTS = res
    out = np.stack([
        np.asarray(res.results[2 * b]["out_p"], dtype=np.float32)
        + np.asarray(res.results[2 * b + 1]["out_p"], dtype=np.float32)
        for b in range(B)
    ])
    return out


# revision 9
# speedup vs baseline: 1.2093x; 1.0036x over previous
"""Bass/Tile kernel for bidirectional multi-head self-attention on 8 trn2 cores.

Problem: x[4, 2048, 1024], W_qkv[3072, 1024], W_proj[1024, 1024], H=16 heads,
Dh=64.  out = proj(softmax(q k^T / sqrt(Dh)) v).

Sharding: core c = (batch b = c//2, head-group g = c%2).  Each core computes
attention for 8 heads of one batch and a full-T partial output projection
(contraction over its 512 C_in columns); host sums the pair partials
(tensor-parallel unshard) and stacks batches.

v3 design:
  - x is pre-transposed on the host (xT [D, T]): phase 1 needs no PE
    transposes or staging copies.
  - scores matmuls for the two heads of a pair sit on partition ranges
    0:64 / 64:128 -> auto tile_position (0,0)/(64,0); issued adjacently
    they run concurrently in separate PE row groups (K=64 would otherwise
    half-fill the array).
  - all elementwise work runs at [128, 1024] grain (the ~300ns fixed
    per-op engine overhead is 30%+ at FD=512): scores PSUM tiles span a
    kt-pair, phase 1/3 PSUM tiles span two 512 outputs.
  - softmax exp is split between ScalarE (exact) and DVE (Schraudolph
    bf16 exp: int16(round(s*A + B)) bit-punned to bf16).
  - ps_y is effectively double buffered (one tag, 4 bufs) so the
    normalization chain (denominator row -> DRAM -> partition-broadcast
    -> reciprocal -> multiply) never stalls the PE: stalls > 3.4us would
    re-throttle the PE clock to 1.2GHz (HAM), which is what sank v2.
  - the output projection runs per q-chunk, overlapping attention.
"""

import os
import numpy as np
import ml_dtypes

import concourse.bass as bass
import concourse.bacc as bacc
import concourse.mybir as mybir
import concourse.tile as tile
from concourse.bass_utils import run_bass_kernel_spmd

# ---- problem constants (hardcoded per harness contract) --------------------
B = 4
T = 2048
D = 1024
H = 16
DH = 64
N_CORES = 8
HPC = H // 2          # heads per core = 8
F = HPC * DH          # 512 = per-core q/k/v feature width

NT = T // 128         # 16 k-tiles
NCC = D // 128        # 8 contraction chunks over D
QC = 512              # q-chunk
NQC = T // QC         # 4 q-chunks

F32 = mybir.dt.float32
BF16 = mybir.dt.bfloat16
I16 = mybir.dt.int16

DT = BF16
NP_DT = ml_dtypes.bfloat16

# Schraudolph bf16 exp approximation: for raw score s, weight is
# exp(s/8) ~= bitcast_bf16(int16(round(s*SCH_A + SCH_B))).
SCH_C = 0.0587
SCH_A = 128.0 * 1.4426950408889634 / 8.0
SCH_B = 128.0 * (127.0 - SCH_C)

# Fraction of exp tiles computed on the DVE (Schraudolph) instead of
# ScalarE: tile idx goes to DVE when (idx * DVE_NUM) % DVE_DEN < DVE_NUM.
DVE_NUM = 3
DVE_DEN = 8

LAST_EXEC_NS = None
LAST_RESULTS = None


def _exp_on_dve(idx):
    return (idx * DVE_NUM) % DVE_DEN < DVE_NUM


def build_program():
    nc = bacc.Bacc()

    xt_d = nc.dram_tensor("x_t", [D, T], DT, kind="ExternalInput")
    wqkv_d = nc.dram_tensor("w_qkv_t", [D, 3 * F], DT, kind="ExternalInput")
    wproj_d = nc.dram_tensor("w_proj_t", [F, D], DT, kind="ExternalInput")
    out_d = nc.dram_tensor("out_p", [T, D], DT, kind="ExternalOutput")

    with tile.TileContext(nc) as tc:
        with (
            tc.tile_pool(name="xw_pool", bufs=1) as xw_pool,
            tc.tile_pool(name="qk_pool", bufs=1) as qk_pool,
            tc.tile_pool(name="v_pool", bufs=1) as v_pool,
            tc.tile_pool(name="y_pool", bufs=1) as y_pool,
            tc.tile_pool(name="wp_pool", bufs=1) as wp_pool,
            tc.tile_pool(name="sc_pool", bufs=1) as sc_pool,
            tc.tile_pool(name="dram_pool", bufs=2, space="DRAM") as dram_pool,
            tc.tile_pool(name="psum", bufs=1, space="PSUM") as psum,
        ):
            # persistent tensors
            xt_sb = [xw_pool.tile([128, T], DT, name=f"xt{cc}")
                     for cc in range(NCC)]
            w_sb = [xw_pool.tile([128, 3 * F], DT, name=f"wqkv{cc}")
                    for cc in range(NCC)]
            # qkT[f]: f 0..3 -> qT for head pair f, f 4..7 -> kT head pair f-4
            qkT = [qk_pool.tile([128, T], DT, name=f"qkT{f}") for f in range(8)]
            # v_aug[i]: two k-tiles [128 t, 2*(8 heads*65)]; col 64 of each
            # head block is 1.0 (softmax denominator via the AV matmul)
            v_aug = [v_pool.tile([128, 2 * HPC * 65], DT, name=f"vaug{i}")
                     for i in range(NT // 2)]
            # yT[hp]: [128 dh (2 heads), T] -- normalized attention output
            yT = [y_pool.tile([128, T], DT, name=f"yT{hp}") for hp in range(4)]
            # W_proj^T slice tiles [128 dh, D]
            wp_sb = [wp_pool.tile([128, D], DT, name=f"wp{i}") for i in range(4)]

            # chunked in consumption order so phase 1 starts ~4us in
            for ck in range(4):
                for cc in range(NCC):
                    nc.sync.dma_start(
                        out=xt_sb[cc][:, ck * 512:(ck + 1) * 512],
                        in_=xt_d[cc * 128:(cc + 1) * 128,
                                 ck * 512:(ck + 1) * 512])
                    nc.sync.dma_start(
                        out=w_sb[cc][:, ck * 384:(ck + 1) * 384],
                        in_=wqkv_d[cc * 128:(cc + 1) * 128,
                                   ck * 384:(ck + 1) * 384])
            for i in range(4):
                nc.sync.dma_start(out=wp_sb[i],
                                  in_=wproj_d[i * 128:(i + 1) * 128, :])

            # ---------------- phase 1: qkv projection ----------------------
            for tcp in range(2):   # t-chunk pairs (1024 t positions)
                t0 = tcp * 1024
                for f in range(8):
                    ps = psum.tile([128, 1024], F32, name="ps_big",
                                   tag="ps_big", bufs=2)
                    for half in range(2):
                        for cc in range(NCC):
                            nc.tensor.matmul(
                                ps[:, half * 512:(half + 1) * 512],
                                lhsT=w_sb[cc][:, f * 128:(f + 1) * 128],
                                rhs=xt_sb[cc][:, t0 + half * 512:
                                              t0 + (half + 1) * 512],
                                start=(cc == 0), stop=(cc == NCC - 1))
                    nc.scalar.activation(
                        qkT[f][:, t0:t0 + 1024], ps,
                        mybir.ActivationFunctionType.Copy)
                for sv in range(4):  # pairs of t-tiles -> one v_aug tile
                    ps = psum.tile([128, 1024], F32, name="ps_big",
                                   tag="ps_big", bufs=2)
                    for half in range(2):
                        tt0 = t0 + sv * 256 + half * 128
                        for cc in range(NCC):
                            nc.tensor.matmul(
                                ps[:, half * 512:(half + 1) * 512],
                                lhsT=xt_sb[cc][:, tt0:tt0 + 128],
                                rhs=w_sb[cc][:, 2 * F:3 * F],
                                start=(cc == 0), stop=(cc == NCC - 1))
                    va = v_aug[tcp * 4 + sv]
                    va_v = va.rearrange("p (k h d) -> p k h d", k=2, h=HPC)
                    nc.vector.tensor_copy(
                        va_v[:, :, :, 0:64],
                        ps.rearrange("p (k h d) -> p k h d", k=2, h=HPC))
                    nc.vector.memset(va_v[:, :, :, 64:65], 1.0)

            # ---------------- phase 2 + 3 interleaved ----------------------
            exp_idx = 0
            d_idx = 0
            # deferred-normalization state: emitted in two spread stages
            # inside the NEXT pair's ktp loop so the DMA round-trip latency
            # never sits at the head of an engine FIFO (head-of-line
            # blocking there stalls queued exps -> PSUM recycling -> PE,
            # and PE idle gaps > 3.4us re-throttle its clock).
            pending = None  # (qc0, hp, ps_y pair)
            staged = None   # (qc0, hp, ps_y pair, d_bc tiles)

            def norm_stage1(task):
                nonlocal d_idx
                _q0, _hp, _ps_y = task
                d_bcs = []
                for hh in range(2):
                    d_sb = sc_pool.tile([65, QC], F32, name="d_sb",
                                        tag="d_sb", bufs=2)
                    if d_idx % 2 == 0:
                        nc.scalar.copy(d_sb[64:65, :], _ps_y[hh][64:65, :])
                    else:
                        nc.vector.tensor_copy(d_sb[64:65, :],
                                              _ps_y[hh][64:65, :])
                    d_idx += 1
                    d_dram = dram_pool.tile([1, QC], F32, name="d_dram",
                                            tag="d_dram")
                    nc.sync.dma_start(out=d_dram, in_=d_sb[64:65, :])
                    d_bc = sc_pool.tile([64, QC], F32, name="d_bc",
                                        tag="d_bc", bufs=2)
                    nc.sync.dma_start(
                        out=d_bc,
                        in_=bass.AP(tensor=d_dram.tensor,
                                    offset=d_dram.offset,
                                    ap=[[0, 64]] + list(d_dram.ap[1:])))
                    d_bcs.append(d_bc)
                return (*task, d_bcs)

            def norm_stage2(task):
                _q0, _hp, _ps_y, d_bcs = task
                for hh in range(2):
                    r_bc = sc_pool.tile([64, QC], F32, name="r_bc",
                                        tag="r_bc", bufs=2)
                    nc.vector.reciprocal_approx_fast(r_bc, d_bcs[hh])
                    y_tmp = sc_pool.tile([64, QC], DT, name="y_tmp",
                                         tag="y_tmp", bufs=2)
                    nc.vector.tensor_mul(y_tmp, _ps_y[hh][0:64, :], r_bc)
                    nc.sync.dma_start(
                        out=yT[_hp][hh * 64:(hh + 1) * 64, _q0:_q0 + QC],
                        in_=y_tmp)

            def emit_proj(qcp):
                for tt in range(qcp * 4, qcp * 4 + 4):
                    ps_o = psum.tile([128, 1024], F32, name="ps_big",
                                     tag="ps_big", bufs=2)
                    for oc in range(2):
                        for hp_ in range(4):
                            nc.tensor.matmul(
                                ps_o[:, oc * 512:(oc + 1) * 512],
                                lhsT=yT[hp_][:, tt * 128:(tt + 1) * 128],
                                rhs=wp_sb[hp_][:, oc * 512:(oc + 1) * 512],
                                start=(hp_ == 0), stop=(hp_ == 3))
                    o_sb = sc_pool.tile([128, D], DT, name="o_sb",
                                        tag="o_sb", bufs=3)
                    nc.scalar.activation(o_sb, ps_o,
                                         mybir.ActivationFunctionType.Copy)
                    nc.sync.dma_start(out=out_d[tt * 128:(tt + 1) * 128, :],
                                      in_=o_sb)

            for qc in range(NQC):
                q0 = qc * QC
                for hp in range(4):
                    qT = qkT[hp]
                    kT = qkT[4 + hp]
                    h0 = 2 * hp
                    ps_y = [psum.tile([65, QC], F32, name="ps_y",
                                      tag="ps_y", bufs=4) for _ in range(2)]

                    def emit_av(task):
                        ktp_, att2_ = task
                        for k2 in range(2):
                            kt = 2 * ktp_ + k2
                            for hh in range(2):
                                nc.tensor.matmul(
                                    ps_y[hh],
                                    lhsT=v_aug[ktp_][:,
                                                     (k2 * HPC + h0 + hh) * 65:
                                                     (k2 * HPC + h0 + hh) * 65
                                                     + 65],
                                    rhs=att2_[hh][:,
                                                  k2 * 512:(k2 + 1) * 512],
                                    start=(kt == 0), stop=(kt == NT - 1))

                    # AV matmuls lag the scores by one ktp group so the exp
                    # latency overlaps PE work instead of stalling it
                    av_task = None
                    for ktp in range(8):
                        if ktp == 1 and pending is not None:
                            staged = norm_stage1(pending)
                            pending = None
                        if ktp == 4 and staged is not None:
                            norm_stage2(staged)
                            staged = None
                        ps2 = []
                        att2 = []
                        for hh in range(2):
                            ps2.append(psum.tile([128, 1024], F32,
                                                 name="ps_big", tag="ps_big",
                                                 bufs=2))
                            att2.append(sc_pool.tile([128, 1024], DT,
                                                     name="att",
                                                     tag=f"att{hh}", bufs=3))
                        # scores for kt pair; adjacent row-group matmuls
                        # (partitions 0:64 vs 64:128) overlap on the PE
                        for k2 in range(2):
                            kt = 2 * ktp + k2
                            for hh in range(2):
                                r0 = hh * 64
                                nc.tensor.matmul(
                                    ps2[hh][:, k2 * 512:(k2 + 1) * 512],
                                    lhsT=kT[r0:r0 + 64,
                                            kt * 128:(kt + 1) * 128],
                                    rhs=qT[r0:r0 + 64, q0:q0 + QC],
                                    start=True, stop=True)
                        for hh in range(2):
                            if _exp_on_dve(exp_idx):
                                nc.vector.tensor_scalar(
                                    out=att2[hh].bitcast(I16),
                                    in0=ps2[hh],
                                    scalar1=SCH_A, scalar2=SCH_B,
                                    op0=mybir.AluOpType.mult,
                                    op1=mybir.AluOpType.add)
                            else:
                                nc.scalar.activation(
                                    att2[hh], ps2[hh],
                                    mybir.ActivationFunctionType.Exp,
                                    scale=1.0 / 8.0)
                            exp_idx += 1
                        if av_task is not None:
                            emit_av(av_task)
                        av_task = (ktp, att2)
                    emit_av(av_task)
                    pending = (q0, hp, ps_y)
                # output projection deferred by one q-chunk
                if qc >= 1:
                    emit_proj(qc - 1)
            # drain: last pair's normalization + last proj chunk
            staged = norm_stage1(pending)
            norm_stage2(staged)
            emit_proj(NQC - 1)
    return nc


_NC_CACHE = None


def _get_program():
    global _NC_CACHE
    if _NC_CACHE is None:
        nc = build_program()
        if not nc.is_finalized():
            nc.finalize()
        _NC_CACHE = nc
    return _NC_CACHE


def make_in_maps(x, W_qkv, W_proj):
    """Shard full inputs into per-core input maps (host-side layout prep)."""
    Wq, Wk, Wv = W_qkv[0:D], W_qkv[D:2 * D], W_qkv[2 * D:3 * D]
    maps = []
    xt_b, wq_g, wp_g = {}, {}, {}
    for b in range(B):
        xt_b[b] = np.ascontiguousarray(x[b].T).astype(NP_DT)
    for g in range(2):
        rows = slice(g * F, (g + 1) * F)
        wq_g[g] = np.ascontiguousarray(
            np.concatenate([Wq[rows].T, Wk[rows].T, Wv[rows].T], axis=1)
        ).astype(NP_DT)
        wp_g[g] = np.ascontiguousarray(W_proj[:, rows].T).astype(NP_DT)
    for core in range(N_CORES):
        b, g = core // 2, core % 2
        maps.append({
            "x_t": xt_b[b],
            "w_qkv_t": wq_g[g],
            "w_proj_t": wp_g[g],
        })
    return maps


def kernel(x, W_qkv, W_proj):
    global LAST_EXEC_NS, LAST_RESULTS
    x = np.asarray(x, dtype=np.float32)
    W_qkv = np.asarray(W_qkv, dtype=np.float32)
    W_proj = np.asarray(W_proj, dtype=np.float32)

    nc = _get_program()
    in_maps = make_in_maps(x, W_qkv, W_proj)
    trace = bool(int(os.environ.get("BASS_KERNEL_TRACE", "0")))
    res = run_bass_kernel_spmd(nc, in_maps, list(range(N_CORES)), trace=trace)
    LAST_EXEC_NS = res.exec_time_ns
    LAST_RESULTS = res
    out = np.stack([
        np.asarray(res.results[2 * b]["out_p"], dtype=np.float32)
        + np.asarray(res.results[2 * b + 1]["out_p"], dtype=np.float32)
        for b in range(B)
    ])
    return out
